# revision 8
# baseline (speedup 1.0000x reference)
"""Trainium2 Bass kernel for the ExemplarBaseline retrieval-kNN model.

Math (per batch b, fully independent across b):
    f      = data.reshape(B*T, CHW) @ W_fe + b_fe            (feature extract)
    d2     = ||f_s - f_t||^2 ; dist = d2**0.25
    sims   = exp(-c * dist)
    numers = 1e-8 + sum_{s<t} sims[s,t] * teach[s, cls]
    score  = numers**gamma / sum_cls ; score[t=0] = 1e-8

Sharding: data-parallel over the batch dim B (128) across 8 NeuronCores,
16 sequences per core.  Host pre-casts x/W to fp8e4m3 and pre-transposes
so the device only does matmuls + a fused epilogue:

  - feats^T [D, tok] = W^T @ x^T as fp8 DoubleRow MMs (2 k-tiles each,
    N=512 so the 256-col DR weight load hides under the MM stream)
  - sq[tok] = ones^T @ (fT*fT)   (diag of the Gram matrix, via PE)
  - per-seq: psum = G - 0.5*(sq_s + sq_t) = -0.5*d2 via 8 bf16 Gram MMs
    plus two bf16 rank-1 MMs adding the sq row/col
  - epilogue: d2 = max(-2*psum, 1e-12) (DVE, also evacuates PSUM);
    dist = exp(0.25*ln d2); sims = exp(-c*dist) (ACT, Ln/Exp only so the
    scalar engine never reloads activation tables); causal mask applied
    as a DVE multiply with a precomputed s<t mask (kills the diagonal);
    numers = smh^T @ teach (bf16); normalize via exp(g*ln(n+eps) - ln den).

Schedule: the per-sequence epilogue MMs of chunk c-1 are interleaved
BETWEEN the feats dt-blocks of chunk c, so the PE executes them without
idling while their DVE/ACT input chains resolve one dt-block earlier.
W is host-reordered dt-major and loaded on the Activation HWDGE queue in
parallel with x on the SP queue, so the PE reaches full rate ~1.5us in.
Post-numers normalization is batched per chunk ([128, 4*10] tiles).

Low-order terms dropped (host-simulated max rel err 8.6e-3 vs 2e-2 gate):
sims bf16 residual MM, teach bf16 residual MM, sq bf16 residual rank-1s.
"""

import numpy as np
import ml_dtypes

B, T, NC = 128, 128, 10
CHW, D = 3072, 1024
NCORES = 8
BL = B // NCORES          # 16 sequences per core
TOK = BL * T              # 2048 tokens per core
KT = CHW // 128           # 24 contraction k-tiles
DT = D // 128             # 8 feature tiles
CHUNKS = [4, 4, 4, 4]     # sequences per pipeline chunk (512 tokens each:
                          # DR weight load (256 cols) only hides at N=512)

EPS_NUMER = 1e-8
EPS_D2 = 1e-12

_NC_CACHE = {}
LAST_RESULTS = None       # BassKernelResults of the most recent run (for test.py)


def _build_bass(use_bias):
    import concourse.mybir as mybir
    import concourse.tile as tile
    from concourse import bacc

    f32 = mybir.dt.float32
    bf16 = mybir.dt.bfloat16
    fp8 = mybir.dt.float8e4
    AF = mybir.ActivationFunctionType
    OP = mybir.AluOpType
    PM = mybir.MatmulPerfMode

    # The ACT table-set chooser picks the FIRST set containing each function:
    # Exp -> set 0, Ln -> set 5, which makes every Ln<->Exp transition reload
    # tables (~1.3us each).  Both live together in natural_log_exp_and_others;
    # hide them from every other set so the chooser lands there once.
    if not getattr(bacc, "_ln_exp_tables_patched", False):
        orig_tables = bacc.get_activation_tables

        def _patched_tables(arch):
            out = {}
            for name, funcs in orig_tables(arch).items():
                if name != "natural_log_exp_and_others":
                    funcs = funcs - {AF.Ln, AF.Exp}
                out[name] = funcs
            return out

        bacc.get_activation_tables = _patched_tables
        bacc._ln_exp_tables_patched = True

    nc = bacc.Bacc("TRN2", target_bir_lowering=False)

    xT_h = nc.dram_tensor("xT", [CHW, TOK], fp8, kind="ExternalInput")
    W_h = nc.dram_tensor("Wt", [DT, 128, KT, 128], fp8, kind="ExternalInput")
    bfe_h = nc.dram_tensor("bfe", [D], f32, kind="ExternalInput")
    teach_h = nc.dram_tensor("teach", [BL, T, NC], bf16, kind="ExternalInput")
    negc_h = nc.dram_tensor("negc", [128, 1], f32, kind="ExternalInput")
    gam_h = nc.dram_tensor("gam", [128, 1], f32, kind="ExternalInput")
    y_h = nc.dram_tensor("y", [BL, T, NC], f32, kind="ExternalOutput")

    xT_r = xT_h.rearrange("(kt p) n -> p kt n", p=128)     # [128, 24, 2048]
    W_r = W_h.rearrange("dt p kt m -> p dt kt m")          # [128, 8, 24, 128]
    bfe_r = bfe_h.rearrange("(dt p) -> p dt", p=128)       # [128, 8]
    teach_r = teach_h.rearrange("b s c -> s b c")          # [128, 16, 10]
    y_r = y_h.rearrange("b t c -> t b c")                  # [128, 16, 10]

    nch = len(CHUNKS)
    b0s = np.cumsum([0] + CHUNKS).tolist()

    with tile.TileContext(nc) as tc:
        with (
            tc.tile_pool(name="cpool", bufs=1) as cpool,
            tc.tile_pool(name="xpool", bufs=2) as xpool,
            tc.tile_pool(name="f2pool", bufs=3) as f2pool,
            tc.tile_pool(name="wpool", bufs=3) as wpool,
            tc.tile_pool(name="spool", bufs=2) as spool,
            tc.tile_pool(name="pfpool", bufs=2, space="PSUM") as pfpool,
            tc.tile_pool(name="psqpool", bufs=2, space="PSUM") as psqpool,
            tc.tile_pool(name="pgpool", bufs=3, space="PSUM") as pgpool,
            tc.tile_pool(name="pnpool", bufs=1, space="PSUM") as pnpool,
        ):
            # ---- persistent tiles -------------------------------------
            W_sb = cpool.tile([128, DT, KT, 128], fp8, name="W_sb")
            teach_sb = cpool.tile([128, BL, NC], bf16, name="teach_sb")
            bfe_sb = cpool.tile([128, DT], f32, name="bfe_sb")
            negc_sb = cpool.tile([128, 1], f32, name="negc_sb")
            gam_sb = cpool.tile([128, 1], f32, name="gam_sb")
            eps_sb = cpool.tile([128, 1], f32, name="eps_sb")
            ones_sb = cpool.tile([128, 1], bf16, name="ones_sb")
            onesrow = cpool.tile([1, TOK], bf16, name="onesrow")
            sqn = cpool.tile([1, TOK], bf16, name="sqn")       # -0.5 * sq
            mask = cpool.tile([128, 128], bf16, name="mask")   # 1 iff s < t
            fT = [
                cpool.tile([128, TOK], bf16, name=f"fT{i}") for i in range(DT)
            ]

            # ---- startup DMAs.  The two HWDGE queues (SP + Activation)
            # share ~330GB/s of HBM bandwidth and each transfer completes
            # ~3us after issue, so order strictly by PE need time:
            # dt-block 0 consumes ALL of xc0 (1.57MB) in its first 2.6us,
            # while W-dt_i is needed only every 2.6us.  Scalar gets the
            # first two W blocks + consts; sync delivers xc0 then the
            # remaining W blocks (which also paces the chunk-1 prefetch).
            if use_bias:
                nc.scalar.dma_start(out=bfe_sb, in_=bfe_r)
            for dt in range(2):
                nc.scalar.dma_start(out=W_sb[:, dt], in_=W_r[:, dt])
            nc.scalar.dma_start(out=negc_sb, in_=negc_h[:, :])
            nc.scalar.dma_start(out=gam_sb, in_=gam_h[:, :])
            nc.scalar.dma_start(out=teach_sb, in_=teach_r)

            CH0 = CHUNKS[0] * T
            xc0 = xpool.tile([128, KT, CH0], fp8, name="xc")
            for k0, k1 in ((0, 2), (2, 8), (8, 16), (16, 24)):
                nc.sync.dma_start(out=xc0[:, k0:k1, :],
                                  in_=xT_r[:, k0:k1, 0:CH0])
            for dt in range(2, DT):
                nc.sync.dma_start(out=W_sb[:, dt], in_=W_r[:, dt])

            nc.vector.memset(ones_sb, 1.0)
            nc.vector.memset(eps_sb, EPS_NUMER)
            nc.vector.memset(onesrow, 1.0)
            ones128 = wpool.tile([128, 128], bf16, name="ones128")
            nc.vector.memset(ones128, 1.0)
            # iota = t - s - 1 >= 0 keeps entries exactly where s < t
            nc.gpsimd.affine_select(
                out=mask, in_=ones128, compare_op=OP.is_ge, fill=0.0,
                base=-1, pattern=[[1, 128]], channel_multiplier=-1,
            )

            state = {"xc0": xc0}

            def feats_block(c, dt, xc, tok0, CH):
                pf = pfpool.tile([128, CH], f32, name="pf")
                for k in range(0, KT, 2):
                    nc.tensor.matmul(
                        pf, W_sb[:, dt, k:k + 2, :], xc[:, k:k + 2, :],
                        start=(k == 0), stop=(k == KT - 2),
                        perf_mode=PM.DoubleRow,
                    )
                fsl = fT[dt][:, tok0:tok0 + CH]
                if use_bias:
                    nc.vector.tensor_scalar(
                        fsl, pf, bfe_sb[:, dt:dt + 1], None, op0=OP.add,
                    )
                else:
                    nc.vector.tensor_copy(fsl, pf)
                f2 = f2pool.tile([128, CH], bf16, name="f2")
                nc.vector.tensor_mul(f2, fsl, fsl)
                return f2

            def emit_psq(c, dt, f2, CH):
                if dt == 0:
                    state[("psq", c)] = psqpool.tile([1, CH], f32, name="psq")
                nc.tensor.matmul(
                    state[("psq", c)], ones_sb, f2,
                    start=(dt == 0), stop=(dt == DT - 1),
                )

            def make_epilogue_items(p, drain=False):
                """Closures emitting chunk p's per-seq epilogue, consumed one
                per feats dt-block of chunk p+1 (or drained at the end)."""
                S = CHUNKS[p]
                b0 = b0s[p]
                tok0 = b0 * T
                f2last, CH = state.pop(("f2last", p))
                smh_tiles = {}

                def item0():
                    emit_psq(p, DT - 1, f2last, CH)
                    psq = state.pop(("psq", p))
                    sqf = wpool.tile([1, CH], f32, name="sqf")
                    nc.vector.tensor_scalar(sqf, psq, -0.5, None, op0=OP.mult)
                    nc.vector.tensor_copy(sqn[0:1, tok0:tok0 + CH], sqf)

                def mk_gram(si):
                    def g():
                        tsl = slice((b0 + si) * T, (b0 + si + 1) * T)
                        pg = pgpool.tile([128, 128], f32, name="pg")
                        for dti in range(DT):
                            nc.tensor.matmul(
                                pg, fT[dti][:, tsl], fT[dti][:, tsl],
                                start=(dti == 0), stop=False,
                            )
                        nc.tensor.matmul(pg, sqn[:, tsl], onesrow[:, tsl],
                                         start=False, stop=False)
                        nc.tensor.matmul(pg, onesrow[:, tsl], sqn[:, tsl],
                                         start=False, stop=True)
                        # d2 = max(-2*psum, eps); also evacuates the PSUM bank
                        d2c = wpool.tile([128, 128], f32, name="d2c")
                        nc.vector.tensor_scalar(
                            d2c, pg, -2.0, EPS_D2, op0=OP.mult, op1=OP.max,
                        )
                        lt = wpool.tile([128, 128], f32, name="lt")
                        nc.scalar.activation(lt, d2c, AF.Ln)
                        dist = wpool.tile([128, 128], f32, name="dist")
                        nc.scalar.activation(dist, lt, AF.Exp, scale=0.25)
                        sims = wpool.tile([128, 128], f32, name="sims")
                        nc.scalar.activation(sims, dist, AF.Exp, scale=negc_sb)
                        smh = wpool.tile([128, 128], bf16, name="smh")
                        # On GpSimd (idle): in the DVE strict FIFO this op's
                        # ~1.4us wait for the ACT chain head-of-line blocks
                        # the next feats evac/square and stalls the psq MM.
                        nc.gpsimd.tensor_mul(smh, sims, mask)
                        smh_tiles[si] = smh
                    return g

                def mk_numers(si):
                    def f():
                        if si == 0:
                            state[("pn", p)] = pnpool.tile(
                                [128, S, NC], f32, name="pn")
                        nc.tensor.matmul(
                            state[("pn", p)][:, si, :], smh_tiles.pop(si),
                            teach_sb[:, b0 + si, :],
                            start=True, stop=True,
                        )
                    return f

                def mk_post(s0, s1):
                    # normalize seqs [s0, s1) of the chunk (pn slices are in
                    # one shared psum tile, so halves need no extra banks)
                    def post():
                        pn = state[("pn", p)]
                        H = s1 - s0
                        l2 = spool.tile([128, H, NC], f32, name="l2")
                        nc.scalar.activation(l2, pn[:, s0:s1, :], AF.Ln,
                                             bias=eps_sb)
                        tmp = spool.tile([128, H, NC], f32, name="tmp")
                        nc.scalar.activation(tmp, l2, AF.Exp, scale=gam_sb)
                        den = spool.tile([128, H], f32, name="den")
                        nc.vector.tensor_reduce(
                            den, tmp, axis=mybir.AxisListType.X, op=OP.add,
                        )
                        rden = spool.tile([128, H], f32, name="rden")
                        nc.vector.reciprocal(rden, den)
                        nld = spool.tile([128, H], f32, name="nld")
                        nc.scalar.activation(nld, rden, AF.Ln)
                        score = spool.tile([128, H, NC], f32, name="score")
                        for si in range(H):
                            # score = tmp/den = exp(g*l2 + ln(1/den))
                            nc.scalar.activation(
                                score[:, si, :], l2[:, si, :], AF.Exp,
                                scale=gam_sb, bias=nld[:, si:si + 1],
                            )
                        nc.vector.memset(score[0:1, :, :], EPS_NUMER)
                        nc.sync.dma_start(
                            out=y_r[:, b0 + s0:b0 + s1, :], in_=score)
                    return post

                half = S // 2
                post_a, post_b = mk_post(0, half), mk_post(half, S)
                if drain:
                    # no following feats to hide chain latency: issue all
                    # gram bursts first so the ACT/DVE chains pipeline
                    items = [item0] + [mk_gram(si) for si in range(S)]
                    ns = [mk_numers(si) for si in range(S)]
                    items += ns[:half] + [lambda: (ns[half](), post_a())]
                    items += ns[half + 1:] + [post_b]
                else:
                    items = [item0, mk_gram(0)]
                    for si in range(1, S):
                        g, n = mk_gram(si), mk_numers(si - 1)
                        if si == half + 1:
                            items.append(lambda g=g, n=n: (n(), post_a(), g()))
                        else:
                            items.append(lambda g=g, n=n: (n(), g()))
                    items.append(mk_numers(S - 1))
                    items.append(post_b)
                return items

            def emit_chunk(c, prev_items):
                S = CHUNKS[c]
                tok0 = b0s[c] * T
                CH = S * T
                xc = state.pop("xc0") if c == 0 else state.pop(("xc", c))
                f2_prev = None
                it = 0
                for dt in range(DT):
                    f2 = feats_block(c, dt, xc, tok0, CH)
                    if dt >= 1:
                        emit_psq(c, dt - 1, f2_prev, CH)
                    f2_prev = f2
                    if prev_items and it < len(prev_items):
                        prev_items[it]()
                        it += 1
                    if c + 1 < nch and dt in (2, 3, 4, 5):
                        CHn = CHUNKS[c + 1] * T
                        tn = b0s[c + 1] * T
                        if dt == 2:
                            state[("xc", c + 1)] = xpool.tile(
                                [128, KT, CHn], fp8, name="xc")
                        xn = state[("xc", c + 1)]
                        k0 = (dt - 2) * 6
                        nc.sync.dma_start(
                            out=xn[:, k0:k0 + 6, :],
                            in_=xT_r[:, k0:k0 + 6, tn:tn + CHn],
                        )
                state[("f2last", c)] = (f2_prev, CH)
                while prev_items and it < len(prev_items):
                    prev_items[it]()
                    it += 1

            prev_items = None
            for c in range(nch):
                emit_chunk(c, prev_items)
                prev_items = make_epilogue_items(c, drain=(c == nch - 1))
            for f in prev_items:
                f()

    nc.compile()
    return nc


def _get_bass(use_bias=False):
    if use_bias not in _NC_CACHE:
        _NC_CACHE[use_bias] = _build_bass(use_bias)
    return _NC_CACHE[use_bias]


def make_in_maps(data_t, teaching_signal_t, W_fe, b_fe, c, gamma):
    """Host-side prep: cast to fp8/bf16, transpose x, reorder W dt-major,
    shard 8 ways."""
    f8 = ml_dtypes.float8_e4m3
    bf = ml_dtypes.bfloat16
    x = np.asarray(data_t, np.float32).reshape(B * T, CHW)
    x8 = x.astype(f8)
    W8 = np.asarray(W_fe, np.float32).astype(f8)
    # W3[dt, p, kt, m] = W[kt*128+p, dt*128+m]: each dt-block is one
    # contiguous [128 x 3KB] DMA and feeds a full feats dt-block
    W3 = np.ascontiguousarray(
        W8.reshape(KT, 128, DT, 128).transpose(2, 1, 0, 3))
    bfe = np.ascontiguousarray(np.asarray(b_fe, np.float32).reshape(D))
    teach = np.asarray(teaching_signal_t, np.float32).astype(bf)
    cval = np.float32(np.asarray(c, np.float32).reshape(-1)[0])
    gval = np.float32(np.asarray(gamma, np.float32).reshape(-1)[0])
    negc = np.full((128, 1), -cval, np.float32)
    gam = np.full((128, 1), gval, np.float32)

    in_maps = []
    for core in range(NCORES):
        rows = slice(core * TOK, (core + 1) * TOK)
        xT_c = np.ascontiguousarray(x8[rows].T)          # [3072, 2048]
        m = dict(
            xT=xT_c, Wt=W3, bfe=bfe,
            teach=np.ascontiguousarray(teach[core * BL:(core + 1) * BL]),
            negc=negc, gam=gam,
        )
        in_maps.append(m)
    return in_maps


def kernel(responses_t, data_t, teaching_signal_t, W_fe, b_fe, c, gamma):
    global LAST_RESULTS
    from concourse.bass_utils import run_bass_kernel_spmd

    use_bias = bool(np.any(np.asarray(b_fe, np.float32)))
    in_maps = make_in_maps(data_t, teaching_signal_t, W_fe, b_fe, c, gamma)
    nc = _get_bass(use_bias)
    res = run_bass_kernel_spmd(nc, in_maps, core_ids=list(range(NCORES)))
    LAST_RESULTS = res
    y = np.concatenate([r["y"] for r in res.results], axis=0)  # [128,128,10]
    return np.ascontiguousarray(y[:, :, None, :].astype(np.float32))
